# revision 6
# baseline (speedup 1.0000x reference)
"""AnyPrecisionLinear (4-bit LUT-quantized linear) on 8 TRN2 NeuronCores.

y = x @ dequant(qweight, lut).T + bias
  x (4, 2048, 4096) fp16, qweight (4, 11008, 128) int32 bitplanes,
  lut (11008, 16) fp16 per-row codebook, bias (11008,) fp16.

Sharding: tensor-parallel on out_features. Each core gets 1376 output
rows (padded to 1408 = 11*128); x is replicated. Each core computes
y_c = x @ W_c.T + b_c fully on device (dequant included); the host only
concatenates the 8 output shards along the feature axis.

Per-core pipeline:
  dequant: bitplane AND-extraction (broadcast word reads) -> int16
    masks + fp16 LSB bit -> 8x tensor_scalar LUT pair-select ->
    7x copy_predicated binary-tree merge -> W (o-part, j-free) fp16
    -> SBUF->SBUF DMA-xbar transpose into resident Wt (j-part, o-free).
  gemm: o-chunk-outer loop so PE work on already-dequantized o-chunks
    overlaps dequant of later ones; x streamed via DMA-transpose loads
    (j-part, t-free); PSUM (128 t, <=512 o) accumulated over j;
    epilogue adds bias (fp16 broadcast row) and casts to fp16.
"""

import numpy as np
from contextlib import ExitStack

import concourse.bass as bass
import concourse.mybir as mybir
import concourse.tile as tile
from concourse import bacc
from concourse import bass_utils

P = 128
N_CORES = 8
O_FULL = 11008
O_REAL = O_FULL // N_CORES       # 1376
OT = 11                          # o-tiles per core (padded)
O_SH = OT * P                    # 1408
IN = 4096
INW = IN // 32                   # 128 int32 words per row
T = 8192
JC = 512                         # j-chunk for dequant (16 words)
NCH = IN // JC                   # 4 chunks
NJT = IN // P                    # 32 j-tiles
TCH = 256                        # token chunk
NTCH = T // TCH                  # 16
OC_SPLITS = [(0, 512), (512, 512), (1024, 384)]  # o-chunks of Wt free dim

A = mybir.AluOpType
DT = mybir.dt

_cached = {}


def build(cfg=None):
    cfg = cfg or {}
    # engine assignment for the 4 AND ops (one per bitplane 0..3)
    and_eng = cfg.get("and_eng", "vvvv")      # v=vector, g=gpsimd
    mask_eng = cfg.get("mask_eng", "vvv")     # planes 0..2 -> int16 masks
    l0_eng = cfg.get("l0_eng", "vvvvvvvv")    # 8 level-0 ops: s=scalar(ACT), v=vector
    epi_eng = cfg.get("epi_eng", "v")
    oc_outer = cfg.get("oc_outer", True)

    nc = bacc.Bacc("TRN2", target_bir_lowering=False, debug=False,
                   num_devices=N_CORES)
    x_d = nc.dram_tensor("x", (T, IN), DT.float16, kind="ExternalInput")
    qw_d = nc.dram_tensor("qw", (4, O_SH, INW), DT.int32, kind="ExternalInput")
    lut_d = nc.dram_tensor("lut", (O_SH, 16), DT.float16, kind="ExternalInput")
    bias_d = nc.dram_tensor("bias", (1, O_SH), DT.float16, kind="ExternalInput")
    onebit_d = nc.dram_tensor("onebit", (1, JC), DT.int32, kind="ExternalInput")
    y_d = nc.dram_tensor("y", (T, O_SH), DT.float16, kind="ExternalOutput")

    def eng(ch):
        return {"v": nc.vector, "g": nc.gpsimd, "s": nc.scalar}[ch]

    with tile.TileContext(nc) as tc, ExitStack() as ctx:
        wt_pool = ctx.enter_context(tc.tile_pool(name="wt", bufs=1))
        const_pool = ctx.enter_context(tc.tile_pool(name="const", bufs=1))

        # resident transposed weights: 32 tiles (128 j, 1408 o) fp16
        wt = [wt_pool.tile([P, O_SH], DT.float16, tag=f"wt{j}", name=f"wt{j}")
              for j in range(NJT)]

        onebit_sb = const_pool.tile([P, JC], DT.int32)
        nc.sync.dma_start(onebit_sb[:], onebit_d.ap().to_broadcast((P, JC)))
        bias_sb = const_pool.tile([P, O_SH], DT.float16)
        nc.sync.dma_start(bias_sb[:], bias_d.ap().to_broadcast((P, O_SH)))

        # ---------------- dequant of one o-tile g ----------------
        def dequant_g(g, dq, dqs):
            qw_sb = dq.tile([P, 4, INW], DT.int32, tag="qw", name="qw")
            nc.sync.dma_start(qw_sb[:], qw_d[:, g * P:(g + 1) * P, :]
                              .rearrange("b o k -> o b k"))
            lut_sb = dqs.tile([P, 16], DT.float16, tag="lut", name="lut")
            nc.sync.dma_start(lut_sb[:], lut_d[g * P:(g + 1) * P, :])
            d_sb = dqs.tile([P, 8], DT.float32, tag="d", name="d")
            nc.vector.tensor_tensor(d_sb[:], lut_sb[:, 1::2], lut_sb[:, 0::2],
                                    A.subtract)
            le_sb = dqs.tile([P, 8], DT.float32, tag="le", name="le")
            nc.vector.tensor_copy(le_sb[:], lut_sb[:, 0::2])

            for c in range(NCH):
                wsl = slice(c * (JC // 32), (c + 1) * (JC // 32))
                and_t = []
                for i in range(4):
                    a = dq.tile([P, JC], DT.int32, tag=f"and{i}", name=f"and{i}")
                    src = qw_sb[:, i, wsl, None].to_broadcast((P, JC // 32, 32))
                    eng(and_eng[i]).tensor_tensor(
                        a[:].rearrange("p (w s) -> p w s", s=32), src,
                        onebit_sb[:].rearrange("p (w s) -> p w s", s=32),
                        A.bitwise_and)
                    and_t.append(a)
                m_t = []
                for i in range(3):
                    m = dq.tile([P, JC], DT.int16, tag=f"m{i}", name=f"m{i}")
                    eng(mask_eng[i]).tensor_scalar(m[:], and_t[i][:], 0, None,
                                                   A.not_equal)
                    m_t.append(m)
                b0 = dq.tile([P, JC], DT.float16, tag="b0", name="b0")
                nc.vector.tensor_scalar(b0[:], and_t[3][:], 0, None, A.not_equal)

                t_t = []
                for k in range(8):
                    t = dq.tile([P, JC], DT.float16, tag=f"t{k}", name=f"t{k}")
                    if l0_eng[k] == "s":
                        nc.scalar.activation(t[:], b0[:],
                                             mybir.ActivationFunctionType.Identity,
                                             bias=le_sb[:, k:k + 1],
                                             scale=d_sb[:, k:k + 1])
                    else:
                        eng(l0_eng[k]).tensor_scalar(
                            t[:], b0[:], d_sb[:, k:k + 1], le_sb[:, k:k + 1],
                            A.mult, A.add)
                    t_t.append(t)
                for m in range(4):
                    nc.vector.copy_predicated(t_t[2 * m][:], m_t[2][:],
                                              t_t[2 * m + 1][:])
                for m in range(2):
                    nc.vector.copy_predicated(t_t[4 * m][:], m_t[1][:],
                                              t_t[4 * m + 2][:])
                nc.vector.copy_predicated(t_t[0][:], m_t[0][:], t_t[4][:])
                # transpose 8 (128,128) blocks into Wt tiles
                for q in range(JC // P):
                    jt = c * (JC // P) + q
                    nc.sync.dma_start_transpose(
                        wt[jt][:, g * P:(g + 1) * P],
                        t_t[0][:, q * P:(q + 1) * P])

        # ---------------- gemm over one o-chunk ----------------
        def gemm_pass(oc_idx, o_lo, o_n, xp, pp, yp):
            for tc_i in range(NTCH):
                xt = []
                for jt in range(NJT):
                    xtile = xp.tile([P, TCH], DT.float16, tag=f"xt{jt}",
                                    name=f"xt{jt}")
                    nc.sync.dma_start_transpose(
                        xtile[:], x_d[tc_i * TCH:(tc_i + 1) * TCH,
                                      jt * P:(jt + 1) * P])
                    xt.append(xtile)
                for ts_i in range(TCH // P):
                    ps = pp.tile([P, 512], DT.float32, tag="ps", name="ps")
                    for jt in range(NJT):
                        nc.tensor.matmul(
                            ps[:, :o_n],
                            xt[jt][:, ts_i * P:(ts_i + 1) * P],
                            wt[jt][:, o_lo:o_lo + o_n],
                            start=(jt == 0), stop=(jt == NJT - 1))
                    ysb = yp.tile([P, 512], DT.float16, tag="y", name="y")
                    eng(epi_eng).tensor_tensor(
                        ysb[:, :o_n], ps[:, :o_n],
                        bias_sb[:, o_lo:o_lo + o_n], A.add)
                    t0 = tc_i * TCH + ts_i * P
                    nc.sync.dma_start(y_d[t0:t0 + P, o_lo:o_lo + o_n],
                                      ysb[:, :o_n])

        with tc.tile_pool(name="dq", bufs=2) as dq, \
             tc.tile_pool(name="dqs", bufs=3) as dqs, \
             tc.tile_pool(name="xp", bufs=2) as xp, \
             tc.tile_pool(name="pp", bufs=4, space="PSUM") as pp, \
             tc.tile_pool(name="yp", bufs=4) as yp:
            if oc_outer:
                # interleave: dequant the g-range of each o-chunk, then its gemm
                done_g = 0
                for oc_idx, (o_lo, o_n) in enumerate(OC_SPLITS):
                    g_hi = (o_lo + o_n + P - 1) // P
                    for g in range(done_g, g_hi):
                        dequant_g(g, dq, dqs)
                    done_g = g_hi
                    gemm_pass(oc_idx, o_lo, o_n, xp, pp, yp)
            else:
                for g in range(OT):
                    dequant_g(g, dq, dqs)
                for oc_idx, (o_lo, o_n) in enumerate(OC_SPLITS):
                    gemm_pass(oc_idx, o_lo, o_n, xp, pp, yp)

    nc.compile()
    return nc


def _onebit():
    shifts = (31 - np.arange(JC) % 32).astype(np.uint32)
    return (np.uint32(1) << shifts).view(np.int32).reshape(1, JC).copy()


def kernel(x, qweight, lut, bias, w_bits, cfg=None, _want_results=False,
           _trace=False):
    assert int(w_bits) == 4
    key = tuple(sorted((cfg or {}).items()))
    if key not in _cached:
        _cached[key] = build(cfg)
    nc = _cached[key]

    x2d = np.ascontiguousarray(x.reshape(T, IN)).astype(np.float16, copy=False)
    onebit = _onebit()
    in_maps = []
    for c in range(N_CORES):
        lo = c * O_REAL
        qw_c = np.zeros((4, O_SH, INW), np.int32)
        qw_c[:, :O_REAL] = qweight[:, lo:lo + O_REAL]
        lut_c = np.zeros((O_SH, 16), np.float16)
        lut_c[:O_REAL] = lut[lo:lo + O_REAL]
        bias_c = np.zeros((1, O_SH), np.float16)
        bias_c[0, :O_REAL] = bias[lo:lo + O_REAL]
        in_maps.append({"x": x2d, "qw": qw_c, "lut": lut_c, "bias": bias_c,
                       "onebit": onebit})

    res = bass_utils.run_bass_kernel_spmd(nc, in_maps,
                                          core_ids=list(range(N_CORES)),
                                          trace=_trace)
    y = np.empty((T, O_FULL), np.float16)
    for c in range(N_CORES):
        y[:, c * O_REAL:(c + 1) * O_REAL] = res.results[c]["y"][:, :O_REAL]
    out = y.reshape(4, 2048, O_FULL)
    if _want_results:
        return out, res
    return out


# revision 10
# speedup vs baseline: 2.3714x; 2.3714x over previous
"""AnyPrecisionLinear (4-bit LUT-quantized linear) on 8 TRN2 NeuronCores.

y = x @ dequant(qweight, lut).T + bias
  x (4,2048,4096) fp16, qweight (4,11008,128) int32 bitplanes (MSB-first),
  lut (11008,16) fp16 per-row codebook, bias (11008,) fp16.

Sharding: tensor-parallel on out_features; 1376 rows/core padded to 1408.
x replicated; host concatenates the 8 (8192,1376) output shards.

Per-core:
  dequant (DVE + ACT + GPS): bitplane AND extraction -> int16 {0,1} masks
    -> 8x level-0 LUT pair select (tensor_scalar / ACT Identity)
    -> 7x copy_predicated tree merge -> W (o,j) fp16 -> DRAM scratch.
  Wt load: 32 big DMA-xbar transpose loads (1408,128)->(128,1408),
    alternated across the two HWDGE engines (sync/scalar).
  gemm: per 512-token chunk, 32 transposed x loads (512,128)->(128,512)
    alternated across HWDGE engines; per 128-token subtile one PSUM bank
    per o-chunk (512/512/384) accumulated over all 32 j-tiles with the
    x tile as the shared stationary operand (3 matmuls per LDWEIGHTS);
    bias applied as a final K=1 matmul; ACT Copy epilogue to fp16.
"""

import numpy as np
from contextlib import ExitStack

import concourse.bass as bass
import concourse.mybir as mybir
import concourse.tile as tile
from concourse import bacc
from concourse import bass_utils

P = 128
N_CORES = 8
O_FULL = 11008
O_REAL = O_FULL // N_CORES       # 1376
OT = 11
O_SH = OT * P                    # 1408
IN = 4096
INW = IN // 32
T = 8192
JC = 512                         # dequant j-chunk (16 words)
NCH = IN // JC
NJT = IN // P                    # 32
TCH = 512
NTCH = T // TCH                  # 16
OC_SPLITS = [(0, 512), (512, 512), (1024, 384)]

A = mybir.AluOpType
DT = mybir.dt
AF = mybir.ActivationFunctionType

_cached = {}


def build(cfg=None):
    cfg = cfg or {}
    l0_eng = cfg.get("l0_eng", "ssggssgg")  # 8 level-0 ops: s=ACT, g=GPS, v=DVE
    cp_bitcast = cfg.get("cp_bitcast", True)
    xt_bufs = cfg.get("xt_bufs", 2)

    nc = bacc.Bacc("TRN2", target_bir_lowering=False, debug=False,
                   num_devices=N_CORES)
    x_d = nc.dram_tensor("x", (T, IN), DT.float16, kind="ExternalInput")
    qw_d = nc.dram_tensor("qw", (4, O_SH, INW), DT.int32, kind="ExternalInput")
    lut_d = nc.dram_tensor("lut", (O_SH, 16), DT.float16, kind="ExternalInput")
    bias_d = nc.dram_tensor("bias", (1, O_SH), DT.float16, kind="ExternalInput")
    onebit_d = nc.dram_tensor("onebit", (1, JC), DT.int32, kind="ExternalInput")
    y_d = nc.dram_tensor("y", (T, O_SH), DT.float16, kind="ExternalOutput")
    debug = (cfg or {}).get("debug", False)
    if debug:
        wdump_d = nc.dram_tensor("wdump", (O_SH, IN), DT.float16,
                                 kind="ExternalOutput")
        wtdump_d = nc.dram_tensor("wtdump", (IN, O_SH), DT.float16,
                                  kind="ExternalOutput")

    def eng(ch):
        return {"v": nc.vector, "g": nc.gpsimd, "s": nc.scalar}[ch]

    hw = [nc.sync, nc.scalar] if cfg.get("two_hwdge", False) else [nc.sync, nc.sync]

    with tile.TileContext(nc) as tc, ExitStack() as ctx:
        wt_pool = ctx.enter_context(tc.tile_pool(name="wt", bufs=1))
        const_pool = ctx.enter_context(tc.tile_pool(name="const", bufs=1))
        dram = ctx.enter_context(tc.tile_pool(name="dram", bufs=1, space="DRAM"))

        wt = [wt_pool.tile([P, O_SH], DT.float16, tag=f"wt{j}", name=f"wt{j}")
              for j in range(NJT)]
        w_dram = dram.tile([O_SH, IN], DT.float16, name="w_dram")

        onebit_sb = const_pool.tile([P, JC], DT.int32)
        nc.sync.dma_start(onebit_sb[:], onebit_d.ap().to_broadcast((P, JC)))
        bias_row = const_pool.tile([1, O_SH], DT.float16)
        nc.sync.dma_start(bias_row[:], bias_d[:, :])
        ones_row = const_pool.tile([1, P], DT.float16)
        nc.vector.memset(ones_row[:], 1.0)

        # ---------------- dequant of one o-tile g ----------------
        def dequant_g(g, dq, dqs):
            qw_sb = dq.tile([P, 4, INW], DT.int32, tag="qw", name="qw")
            nc.sync.dma_start(qw_sb[:], qw_d[:, g * P:(g + 1) * P, :]
                              .rearrange("b o k -> o b k"))
            lut_sb = dqs.tile([P, 16], DT.float16, tag="lut", name="lut")
            nc.sync.dma_start(lut_sb[:], lut_d[g * P:(g + 1) * P, :])
            d_sb = dqs.tile([P, 8], DT.float32, tag="d", name="d")
            nc.vector.tensor_tensor(d_sb[:], lut_sb[:, 1::2], lut_sb[:, 0::2],
                                    A.subtract)
            le_sb = dqs.tile([P, 8], DT.float32, tag="le", name="le")
            nc.vector.tensor_copy(le_sb[:], lut_sb[:, 0::2])

            for c in range(NCH):
                wsl = slice(c * (JC // 32), (c + 1) * (JC // 32))
                m_t = []
                b0 = None
                for i in range(4):
                    a = dq.tile([P, JC], DT.int32, tag="and", name="anda")
                    src = qw_sb[:, i, wsl, None].to_broadcast((P, JC // 32, 32))
                    nc.vector.tensor_tensor(
                        a[:].rearrange("p (w s) -> p w s", s=32), src,
                        onebit_sb[:].rearrange("p (w s) -> p w s", s=32),
                        A.bitwise_and)
                    if i < 3:
                        m = dq.tile([P, JC], DT.int16, tag=f"m{i}", name=f"m{i}")
                        nc.vector.tensor_scalar(m[:], a[:], 0, None, A.not_equal)
                        m_t.append(m)
                    else:
                        b0 = dq.tile([P, JC], DT.float16, tag="b0", name="b0")
                        nc.vector.tensor_scalar(b0[:], a[:], 0, None, A.not_equal)

                t_t = []
                for k in range(8):
                    t = dq.tile([P, JC], DT.float16, tag=f"t{k}", name=f"t{k}")
                    if l0_eng[k] == "s":
                        nc.scalar.activation(t[:], b0[:], AF.Identity,
                                             bias=le_sb[:, k:k + 1],
                                             scale=d_sb[:, k:k + 1])
                    else:
                        eng(l0_eng[k]).tensor_scalar(
                            t[:], b0[:], d_sb[:, k:k + 1], le_sb[:, k:k + 1],
                            A.mult, A.add)
                    t_t.append(t)

                def merge(dst, msk, src):
                    if cp_bitcast:
                        nc.vector.copy_predicated(
                            dst[:].bitcast(DT.int16), msk[:],
                            src[:].bitcast(DT.int16))
                    else:
                        nc.vector.copy_predicated(dst[:], msk[:], src[:])

                for m in range(4):
                    merge(t_t[2 * m], m_t[2], t_t[2 * m + 1])
                for m in range(2):
                    merge(t_t[4 * m], m_t[1], t_t[4 * m + 2])
                merge(t_t[0], m_t[0], t_t[4])
                nc.sync.dma_start(w_dram[g * P:(g + 1) * P, c * JC:(c + 1) * JC],
                                  t_t[0][:])

        # ---------------- build ----------------
        with tc.tile_pool(name="dq", bufs=2) as dq, \
             tc.tile_pool(name="dqs", bufs=3) as dqs, \
             tc.tile_pool(name="xp", bufs=xt_bufs) as xp, \
             tc.tile_pool(name="pp", bufs=6, space="PSUM") as pp, \
             tc.tile_pool(name="yp", bufs=4) as yp:

            for g in range(OT):
                dequant_g(g, dq, dqs)

            # transposed weight loads (big xbar transposes, 2 engines)
            for jt in range(NJT):
                hw[jt % 2].dma_start_transpose(
                    wt[jt][:], w_dram[:, jt * P:(jt + 1) * P])
            if debug:
                nc.sync.dma_start(wdump_d[:, :], w_dram[:, :])
                for jt in range(NJT):
                    nc.sync.dma_start(wtdump_d[jt * P:(jt + 1) * P, :], wt[jt][:])

            for tc_i in range(NTCH):
                xt = []
                for jt in range(NJT):
                    xtile = xp.tile([P, TCH], DT.float16, tag=f"xt{jt}",
                                    name=f"xt{jt}")
                    hw[jt % 2].dma_start_transpose(
                        xtile[:], x_d[tc_i * TCH:(tc_i + 1) * TCH,
                                      jt * P:(jt + 1) * P])
                    xt.append(xtile)
                for ts_i in range(TCH // P):
                    pss = [pp.tile([P, 512], DT.float32, tag="ps", name="ps")
                           [:, :o_n] for (_, o_n) in OC_SPLITS]
                    for jt in range(NJT):
                        xsl = xt[jt][:, ts_i * P:(ts_i + 1) * P]
                        for oc, (o_lo, o_n) in enumerate(OC_SPLITS):
                            nc.tensor.matmul(pss[oc], xsl,
                                             wt[jt][:, o_lo:o_lo + o_n],
                                             start=(jt == 0), stop=False)
                    t0 = tc_i * TCH + ts_i * P
                    for oc, (o_lo, o_n) in enumerate(OC_SPLITS):
                        nc.tensor.matmul(pss[oc], ones_row[:, :],
                                         bias_row[:, o_lo:o_lo + o_n],
                                         start=False, stop=True)
                        ysb = yp.tile([P, 512], DT.float16, tag="y", name="y")
                        nc.scalar.activation(ysb[:, :o_n], pss[oc], AF.Copy)
                        nc.sync.dma_start(y_d[t0:t0 + P, o_lo:o_lo + o_n],
                                          ysb[:, :o_n])

    nc.compile()
    return nc


def _onebit():
    shifts = (31 - np.arange(JC) % 32).astype(np.uint32)
    return (np.uint32(1) << shifts).view(np.int32).reshape(1, JC).copy()


def kernel(x, qweight, lut, bias, w_bits, cfg=None, _want_results=False,
           _trace=False):
    assert int(w_bits) == 4
    key = tuple(sorted((cfg or {}).items()))
    if key not in _cached:
        _cached[key] = build(cfg)
    nc = _cached[key]

    x2d = np.ascontiguousarray(x.reshape(T, IN)).astype(np.float16, copy=False)
    onebit = _onebit()
    in_maps = []
    for c in range(N_CORES):
        lo = c * O_REAL
        qw_c = np.zeros((4, O_SH, INW), np.int32)
        qw_c[:, :O_REAL] = qweight[:, lo:lo + O_REAL]
        lut_c = np.zeros((O_SH, 16), np.float16)
        lut_c[:O_REAL] = lut[lo:lo + O_REAL]
        bias_c = np.zeros((1, O_SH), np.float16)
        bias_c[0, :O_REAL] = bias[lo:lo + O_REAL]
        in_maps.append({"x": x2d, "qw": qw_c, "lut": lut_c, "bias": bias_c,
                       "onebit": onebit})

    res = bass_utils.run_bass_kernel_spmd(nc, in_maps,
                                          core_ids=list(range(N_CORES)),
                                          trace=_trace)
    y = np.empty((T, O_FULL), np.float16)
    for c in range(N_CORES):
        y[:, c * O_REAL:(c + 1) * O_REAL] = res.results[c]["y"][:, :O_REAL]
    out = y.reshape(4, 2048, O_FULL)
    if _want_results:
        return out, res
    return out
